# revision 1
# baseline (speedup 1.0000x reference)
"""BiLSTM tagger kernel for 8 Trainium2 NeuronCores.

Strategy: data-parallel over batch (16 sequences per core, weights
replicated). Per core, the two directions of each BiLSTM layer run as
interleaved scans so gate math on ScalarE/VectorE hides under the other
scan's recurrent matmul on TensorE. All matmuls run in bf16 (fp32 matmul
is 4x slower on TRN2); PSUM accumulation stays fp32.

Recurrent step layout: stationary = h^T chunks [128,16], moving = W_hh^T
slices, psum gates [16, 2048]. gx (input projections, precomputed per
layer into HBM) is added on VectorE during the psum drain. h is
re-transposed each step with four tiny matmuls against a 16x16 identity.
Backward scans consume inputs pre-reversed per sequence length (host
permutation indices + indirect DMA); their outputs are scattered back
through the same permutation, which also writes the zero padding the
reference produces. The permutation is t -> len-1-t for t < len, else
t -> t; steps past len compute garbage that is masked to zero and cannot
contaminate earlier steps.
"""

import sys

for _p in ("/opt/trn_rl_repo",):
    if _p not in sys.path:
        sys.path.append(_p)

import numpy as np
import ml_dtypes

import concourse.bass as bass
import concourse.tile as tile
from concourse import bacc, mybir
from concourse.bass import IndirectOffsetOnAxis
from concourse.bass_utils import run_bass_kernel_spmd

F32 = mybir.dt.float32
BF16 = mybir.dt.bfloat16
I32 = mybir.dt.int32
AF = mybir.ActivationFunctionType
ALU = mybir.AluOpType

# problem sizes (full / per-core)
B, T, V, E, H, TAGS = 128, 512, 50000, 256, 512, 64
NC = 8
BL = B // NC   # 16 sequences per core
G = 4 * H      # 2048 gate width

ABLATE = set()  # dev knob: {"gxdma","gates","ring","trans","mm"}

# permutation taking pytorch gate order i,f,g,o -> i,f,o,g (sigmoid block first)
_GATE_PERM = np.concatenate([
    np.arange(0, H), np.arange(H, 2 * H), np.arange(3 * H, 4 * H),
    np.arange(2 * H, 3 * H)])


def _build(nc, Tn=T, Bl=BL, TC=2, RC=4):
    """Emit the per-core program. Tn shrinkable for dev testing."""
    ntok = Bl * Tn
    nchunk = ntok // 128
    KE = E // 128       # k-chunks for layer-1 input proj
    KH2 = 2 * H // 128  # k-chunks for layer-2 input proj / classifier
    KH = H // 128       # k-chunks for recurrent
    assert ntok % 128 == 0

    # ---- dram I/O ----
    emb = nc.dram_tensor("emb", [V, E], F32, kind="ExternalInput")
    xf_idx = nc.dram_tensor("xf_idx", [128, nchunk], I32, kind="ExternalInput")
    xb_idx = nc.dram_tensor("xb_idx", [128, nchunk], I32, kind="ExternalInput")
    rev128 = nc.dram_tensor("rev128", [128, nchunk], I32, kind="ExternalInput")
    rev16 = nc.dram_tensor("rev16", [Bl, Tn], I32, kind="ExternalInput")
    mask = nc.dram_tensor("mask", [Bl, Tn], F32, kind="ExternalInput")
    ident = nc.dram_tensor("ident", [16, 16], BF16, kind="ExternalInput")

    wih, whh, biasd = {}, {}, {}
    for s, din in (("f1", E), ("b1", E), ("f2", 2 * H), ("b2", 2 * H)):
        wih[s] = nc.dram_tensor(f"wihT_{s}", [din, G], BF16, kind="ExternalInput")
        whh[s] = nc.dram_tensor(f"whhT_{s}", [H, G], BF16, kind="ExternalInput")
        biasd[s] = nc.dram_tensor(f"bias_{s}", [128, G], F32, kind="ExternalInput")
    wcls = nc.dram_tensor("wclsT", [2 * H, TAGS], BF16, kind="ExternalInput")
    bcls = nc.dram_tensor("bcls", [TAGS, 1], F32, kind="ExternalInput")

    gx = {s: nc.dram_tensor(f"gx_{s}", [ntok, G], BF16)
          for s in ("f1", "b1", "f2", "b2")}
    # per-direction layer outputs; backward halves stay in scan order and are
    # un-reversed by the consumers' row gathers (no per-step scatters)
    hout = {s: nc.dram_tensor(f"hout_{s}", [ntok, H], BF16)
            for s in ("f1", "b1", "f2", "b2")}
    logitsT = nc.dram_tensor("logitsT", [TAGS, ntok], F32, kind="ExternalOutput")

    with tile.TileContext(nc) as tc:
        with tc.tile_pool(name="const", bufs=1) as cpool:
            def load_const(nm, shape, dt, src_ap):
                t = cpool.tile(shape, dt, name=nm, tag=nm)
                nc.gpsimd.dma_start(t[:], src_ap)
                return t

            xf_sb = load_const("xf_sb", [128, nchunk], I32, xf_idx[:])
            xb_sb = load_const("xb_sb", [128, nchunk], I32, xb_idx[:])
            rev128_sb = load_const("rev128_sb", [128, nchunk], I32, rev128[:])
            rev16_sb = load_const("rev16_sb", [Bl, Tn], I32, rev16[:])
            mask_sb = load_const("mask_sb", [Bl, Tn], F32, mask[:])
            id_sb = load_const("id_sb", [16, 16], BF16, ident[:])
            bcls_sb = load_const("bcls_sb", [TAGS, 1], F32, bcls[:])
            bias_sb = {s: load_const(f"bias_sb_{s}", [128, G], F32, biasd[s][:])
                       for s in ("f1", "b1", "f2", "b2")}
            wcls_sb = cpool.tile([128, KH2, TAGS], BF16, name="wcls_sb")
            for k in range(KH2):
                nc.gpsimd.dma_start(wcls_sb[:, k, :], wcls[128 * k:128 * (k + 1), :])

            # layer-1 input projections (inputs gathered from embedding table)
            _proj_phase(nc, tc, nchunk, KE, wih=wih, bias_sb=bias_sb, gx=gx,
                        jobs=[("f1", emb, xf_sb, True), ("b1", emb, xb_sb, True)])
            # layer-1 scans
            _scan_phase(nc, tc, Tn, Bl, TC, RC, KH,
                        scans=("f1", "b1"), whh=whh, gx=gx, hout=hout,
                        mask_sb=mask_sb, id_sb=id_sb)
            # layer-2 input projections: input token (b,t) for the fwd scan is
            # [f1h[t], s1h[rev(t)]]; for the bwd scan it is [f1h[rev(t)], s1h[t]]
            _proj_phase(nc, tc, nchunk, KH2, wih=wih, bias_sb=bias_sb, gx=gx,
                        jobs=[("f2", (hout["f1"], None, hout["b1"], rev128_sb), None, False),
                              ("b2", (hout["f1"], rev128_sb, hout["b1"], None), None, False)])
            # layer-2 scans
            _scan_phase(nc, tc, Tn, Bl, TC, RC, KH,
                        scans=("f2", "b2"), whh=whh, gx=gx, hout=hout,
                        mask_sb=mask_sb, id_sb=id_sb)

            # classifier: logits^T = W_cls @ out2^T + b_cls
            with tc.tile_pool(name="cls", bufs=3) as gp, \
                 tc.tile_pool(name="clsT", bufs=3) as gtp, \
                 tc.tile_pool(name="clsps", bufs=4, space="PSUM") as pp, \
                 tc.tile_pool(name="clso", bufs=3) as op:
                for c in range(nchunk):
                    o2 = gp.tile([128, 2 * H], BF16, tag="in")
                    nc.gpsimd.dma_start(o2[:, 0:H], hout["f2"][128 * c:128 * (c + 1), :])
                    nc.gpsimd.indirect_dma_start(
                        out=o2[:, H:2 * H], out_offset=None, in_=hout["b2"][:],
                        in_offset=IndirectOffsetOnAxis(ap=rev128_sb[:, c:c + 1], axis=0))
                    o2T = gtp.tile([128, KH2, 128], BF16, tag="inT")
                    for k in range(KH2):
                        nc.sync.dma_start_transpose(
                            o2T[:, k, :], o2[:, 128 * k:128 * (k + 1)])
                    ps = pp.tile([TAGS, 128], F32, name="clsps_t")
                    for k in range(KH2):
                        nc.tensor.matmul(ps[:], wcls_sb[:, k, :], o2T[:, k, :],
                                         start=(k == 0), stop=(k == KH2 - 1))
                    lg = op.tile([TAGS, 128], F32, tag="lg")
                    nc.scalar.activation(lg[:], ps[:], AF.Identity,
                                         bias=bcls_sb[:, 0:1])
                    nc.gpsimd.dma_start(logitsT[:, 128 * c:128 * (c + 1)], lg[:])
    return nc


def _proj_phase(nc, tc, nchunk, KD, wih, bias_sb, gx, jobs):
    """gx_s = input @ W_ih_s^T + b_s, written contiguously in scan-time order.

    jobs: (scan_name, dram_src, idx_tile_or_None, is_emb). For is_emb the idx
    tile holds embedding row ids (fp32 gather + cast); otherwise rows of src
    are read contiguously (idx None) or gathered (idx set, layer-2 backward).
    """
    D = KD * 128
    with tc.tile_pool(name="pw", bufs=1) as wpool, \
         tc.tile_pool(name="pg", bufs=3) as gpool, \
         tc.tile_pool(name="pgT", bufs=3) as tpool, \
         tc.tile_pool(name="pps", bufs=4, space="PSUM") as ppool, \
         tc.tile_pool(name="pout", bufs=3) as opool:
        wsb = {}
        for s, _, _, _ in jobs:
            wsb[s] = wpool.tile([128, KD, G], BF16, tag=f"w{s}", name=f"wih_{s}")
            for k in range(KD):
                nc.gpsimd.dma_start(wsb[s][:, k, :], wih[s][128 * k:128 * (k + 1), :])
        for c in range(nchunk):
            for s, dsrc, idx, is_emb in jobs:
                if is_emb:
                    e32 = gpool.tile([128, D], F32, tag="e32")
                    nc.gpsimd.indirect_dma_start(
                        out=e32[:], out_offset=None, in_=dsrc[:],
                        in_offset=IndirectOffsetOnAxis(ap=idx[:, c:c + 1], axis=0))
                    xin = gpool.tile([128, D], BF16, tag="e16")
                    nc.vector.tensor_copy(xin[:], e32[:])
                else:
                    fsrc, fidx, bsrc, bidx = dsrc
                    xin = gpool.tile([128, D], BF16, tag="e16")
                    for src_t, sidx, lo in ((fsrc, fidx, 0), (bsrc, bidx, H)):
                        if sidx is None:
                            nc.gpsimd.dma_start(xin[:, lo:lo + H],
                                                src_t[128 * c:128 * (c + 1), :])
                        else:
                            nc.gpsimd.indirect_dma_start(
                                out=xin[:, lo:lo + H], out_offset=None, in_=src_t[:],
                                in_offset=IndirectOffsetOnAxis(ap=sidx[:, c:c + 1], axis=0))
                xT = tpool.tile([128, KD, 128], BF16, tag="xT")
                for k in range(KD):
                    nc.sync.dma_start_transpose(
                        xT[:, k, :], xin[:, 128 * k:128 * (k + 1)])
                gout = opool.tile([128, G], BF16, tag="gout")
                for n in range(G // 512):
                    ps = ppool.tile([128, 512], F32, name="pps")
                    for k in range(KD):
                        nc.tensor.matmul(
                            ps[:], xT[:, k, :], wsb[s][:, k, 512 * n:512 * (n + 1)],
                            start=(k == 0), stop=(k == KD - 1))
                    nc.vector.tensor_tensor(
                        out=gout[:, 512 * n:512 * (n + 1)], in0=ps[:],
                        in1=bias_sb[s][:, 512 * n:512 * (n + 1)],
                        op=ALU.add)
                nc.gpsimd.dma_start(gx[s][128 * c:128 * (c + 1), :], gout[:])


def _scan_phase(nc, tc, Tn, Bl, TC, RC, KH, scans, whh, gx, hout,
                mask_sb, id_sb):
    """Software-pipelined gx injection: next step's gx lands in PSUM via
    identity matmuls during this step's idle PE window; recurrent matmuls
    then accumulate onto it (start=False) and ScalarE reads gates straight
    from PSUM. Gates live in two 2-bank halves (A: i,f / B: o,g) so slots
    free as soon as their sigmoid/tanh reads finish."""
    gxv = {s: gx[s].ap().rearrange("(b t) d -> b t d", b=Bl) for s in scans}
    houtv = {s: hout[s].ap().rearrange("(b t) d -> b t d", b=Bl) for s in scans}
    H2 = 2 * H
    with tc.tile_pool(name="sw", bufs=1) as wpool, \
         tc.tile_pool(name="sgx", bufs=4) as gxpool, \
         tc.tile_pool(name="sst", bufs=1) as stpool, \
         tc.tile_pool(name="sps", bufs=4, space="PSUM") as pspool, \
         tc.tile_pool(name="swk", bufs=3) as wkpool, \
         tc.tile_pool(name="shT", bufs=3) as htpool, \
         tc.tile_pool(name="srng", bufs=3) as rpool:
        wsb, c_st, hT = {}, {}, {}
        for s in scans:
            wsb[s] = wpool.tile([128, KH, G], BF16, tag=f"whh{s}", name=f"whh_{s}")
            for k in range(KH):
                nc.gpsimd.dma_start(wsb[s][:, k, :], whh[s][128 * k:128 * (k + 1), :])
            c_st[s] = stpool.tile([Bl, H], F32, tag=f"c{s}", name=f"c_{s}")
            nc.vector.memset(c_st[s][:], 0.0)
            hT[s] = htpool.tile([128, KH * Bl], BF16, tag="hT", name="hT0")
            nc.vector.memset(hT[s][:], 0.0)
        gxc = {s: None for s in scans}
        gA = {s: None for s in scans}
        gB = {s: None for s in scans}
        ring = {s: None for s in scans}

        def load_gx(tt):
            for s in scans:
                gxc[s] = gxpool.tile([Bl, TC, G], BF16, tag="gx", name="gxc")
                nc.gpsimd.dma_start(gxc[s][:], gxv[s][:, tt:tt + TC, :])

        def inject(tt, only=None):
            # psum halves for step tt, pre-filled with gx via identity matmuls
            for s in (scans if only is None else [only]):
                gA[s] = pspool.tile([Bl, H2], F32, tag="ps", name="gA")
                gB[s] = pspool.tile([Bl, H2], F32, tag="ps", name="gB")
                for half, lo in ((gA[s], 0), (gB[s], H2)):
                    for n in range(2):
                        nc.tensor.matmul(
                            half[:, 512 * n:512 * (n + 1)], id_sb[:],
                            gxc[s][:, tt % TC, lo + 512 * n:lo + 512 * (n + 1)],
                            start=True, stop=False, skip_group_check=True)

        load_gx(0)
        inject(0)
        for t in range(Tn):
            # recurrent matmuls accumulate onto the injected gx; ScalarE reads
            # gates from PSUM as each half-group completes
            gact = {}
            for s in scans:
                # A half: i (cols 0:512), f (512:1024); B half: o, g
                for half, cols in ((gA[s], (0, 1)), (gB[s], (3, 2))):
                    for n in cols:
                        dst = half[:, 512 * (n % 2):512 * (n % 2 + 1)]
                        for k in range(KH):
                            nc.tensor.matmul(dst,
                                             hT[s][:, Bl * k:Bl * (k + 1)],
                                             wsb[s][:, k, 512 * n:512 * (n + 1)],
                                             start=False, stop=(k == KH - 1),
                                             skip_group_check=True)
            for s in scans:
                gact[s] = wkpool.tile([Bl, G], F32, tag="gact", name="gact")
                if t % RC == 0:
                    ring[s] = rpool.tile([Bl, RC, H], BF16, tag="ring", name="ring")
            for s in scans:
                nc.scalar.activation(gact[s][:, 0:H2], gA[s][:], AF.Sigmoid)
            for s in scans:
                nc.scalar.activation(gact[s][:, 3 * H:G], gB[s][:, H:H2], AF.Tanh)
            for s in scans:
                nc.scalar.activation(gact[s][:, H2:3 * H], gB[s][:, 0:H], AF.Sigmoid)
            t1, t2, tch, h16 = {}, {}, {}, {}
            for s in scans:
                t1[s] = wkpool.tile([Bl, H], F32, tag="t1", name="t1")
                nc.vector.tensor_tensor(out=t1[s][:], in0=gact[s][:, H:H2],
                                        in1=c_st[s][:], op=ALU.mult)
            for s in scans:
                t2[s] = wkpool.tile([Bl, H], F32, tag="t2", name="t2")
                nc.vector.tensor_tensor(out=t2[s][:], in0=gact[s][:, 0:H],
                                        in1=gact[s][:, 3 * H:G], op=ALU.mult)
            for s in scans:
                nc.vector.tensor_tensor(out=c_st[s][:], in0=t1[s][:], in1=t2[s][:],
                                        op=ALU.add)
            for s in scans:
                tch[s] = wkpool.tile([Bl, H], F32, tag="tch", name="tch")
                nc.scalar.activation(tch[s][:], c_st[s][:], AF.Tanh)
            for s in scans:
                h16[s] = wkpool.tile([Bl, H], BF16, tag="h16", name="h16")
                nc.vector.tensor_tensor(out=h16[s][:], in0=gact[s][:, H2:3 * H],
                                        in1=tch[s][:], op=ALU.mult)
            # allocate transpose psum tiles first (keeps the proven slot
            # rotation), then emit next step's gx injects BEFORE the transpose
            # matmuls so they fill the PE window spent waiting for h16
            hT_ps = {}
            for s in scans:
                hT_ps[s] = pspool.tile([128, KH * Bl], F32, tag="ps", name="hT_ps")
            if t + 1 < Tn:
                if (t + 1) % TC == 0:
                    load_gx(t + 1)
                inject(t + 1, only=scans[0])
            for s in scans:
                for k in range(KH):
                    nc.tensor.matmul(hT_ps[s][:, Bl * k:Bl * (k + 1)],
                                     h16[s][:, 128 * k:128 * (k + 1)], id_sb[:],
                                     start=True, stop=True)
                hTn = htpool.tile([128, KH * Bl], BF16, tag="hT", name="hTn")
                nc.scalar.activation(hTn[:], hT_ps[s][:], AF.Copy)
                hT[s] = hTn
            if t + 1 < Tn:
                inject(t + 1, only=scans[1])
            for s in scans:
                nc.vector.tensor_scalar_mul(ring[s][:, t % RC, :], h16[s][:],
                                            mask_sb[:, t:t + 1])
                if (t + 1) % RC == 0:
                    t0r = t + 1 - RC
                    nc.gpsimd.dma_start(houtv[s][:, t0r:t0r + RC, :], ring[s][:])


def _prep_inputs(inputs, Tn=T, Bl=BL, ncores=NC):
    """Host-side sharding + weight preprocessing. Returns per-core in_maps."""
    x = np.asarray(inputs["x"]).astype(np.int32)
    lengths = np.asarray(inputs["lengths"]).astype(np.int32)
    emb = np.asarray(inputs["emb"], dtype=np.float32)
    ntok = Bl * Tn

    com = {"emb": emb, "ident": np.eye(16, dtype=ml_dtypes.bfloat16)}
    for s in ("f1", "b1", "f2", "b2"):
        w_ih = np.asarray(inputs[f"W_ih_{s}"], np.float32)[_GATE_PERM]
        w_hh = np.asarray(inputs[f"W_hh_{s}"], np.float32)[_GATE_PERM]
        b = np.asarray(inputs[f"b_{s}"], np.float32)[_GATE_PERM]
        com[f"wihT_{s}"] = np.ascontiguousarray(w_ih.T).astype(ml_dtypes.bfloat16)
        com[f"whhT_{s}"] = np.ascontiguousarray(w_hh.T).astype(ml_dtypes.bfloat16)
        com[f"bias_{s}"] = np.tile(b.reshape(1, G), (128, 1))
    com["wclsT"] = np.ascontiguousarray(
        np.asarray(inputs["W_cls"], np.float32).T).astype(ml_dtypes.bfloat16)
    com["bcls"] = np.asarray(inputs["b_cls"], np.float32).reshape(TAGS, 1)

    def chunked(a):  # [ntok] -> [128, ntok//128] with chunk c in column c
        return np.ascontiguousarray(a.reshape(-1).reshape(ntok // 128, 128).T)

    in_maps = []
    for c in range(ncores):
        xs = x[Bl * c:Bl * (c + 1), :Tn]
        ls = np.minimum(lengths[Bl * c:Bl * (c + 1)], Tn)
        ts = np.arange(Tn)[None, :]
        rev = np.where(ts < ls[:, None], ls[:, None] - 1 - ts, ts)  # [Bl,Tn]
        xrev = np.take_along_axis(xs, rev, axis=1)
        flat_rev = (np.arange(Bl)[:, None] * Tn + rev).astype(np.int32)
        m = {
            "xf_idx": chunked(xs),
            "xb_idx": chunked(xrev),
            "rev128": chunked(flat_rev),
            "rev16": np.ascontiguousarray(flat_rev),
            "mask": (ts < ls[:, None]).astype(np.float32),
        }
        m.update(com)
        in_maps.append(m)
    return in_maps


_CACHED = {}


def kernel(**inputs) -> np.ndarray:
    if "nc" not in _CACHED:
        nc = bacc.Bacc("TRN2", target_bir_lowering=False, debug=False,
                       num_devices=NC)
        _build(nc)
        nc.compile()
        _CACHED["nc"] = nc
    nc = _CACHED["nc"]
    in_maps = _prep_inputs(inputs)
    res = run_bass_kernel_spmd(nc, in_maps, core_ids=list(range(NC)), trace=False)
    outs = []
    for c in range(NC):
        lt = res.results[c]["logitsT"]  # [TAGS, ntok]
        outs.append(np.ascontiguousarray(lt.T.reshape(BL, T, TAGS)))
    return np.concatenate(outs, axis=0).astype(np.float32)



# revision 22
# speedup vs baseline: 1.3326x; 1.3326x over previous
"""BiLSTM tagger kernel for 8 Trainium2 NeuronCores.

Strategy: data-parallel over batch (16 sequences per core, weights
replicated). Per core, the two directions of each BiLSTM layer run as one
lane-packed scan: forward lanes on PSUM/SBUF partitions 0:16, backward on
32:48 (matmul tile_position quantizes output partition offsets to 32, so
lanes 16:32 are initialized-once dont-cares). Every elementwise/activation
instruction covers both directions at once; engine cost scales with free
size only, so the packing halves ScalarE/VectorE work vs per-direction
tiles.

Recurrent matmuls run in fp8(e4m3) DoubleRow mode (2 k-tiles per matmul,
0.5 cycles/row): stationary = hT fp8 chunks [128, 2, 16], moving = W_hh^T
fp8 [128, 2, 512], accumulating onto gates PSUM pre-filled with the input
projections gx via a single shared identity matmul per 512-column block
(one id48 stationary covers both directions). Scaling: W_hh pre-scaled by
sw=240/absmax on host, h scaled by sh=64 in the PSUM->SBUF fp8 downcast;
gx (and so W_ih/bias) pre-scaled by sw*sh on host; the gate activations
descale with a per-partition scale AP. Gate activations are stored bf16
(enables DVE 2-byte fast modes); c stays f32.

gx (input projections) are precomputed per layer into HBM by a standard
m=128 projection phase. Backward scans consume inputs pre-reversed per
sequence length (host permutation indices + indirect DMA); outputs stay in
scan order and consumers un-reverse via the same indices. Steps past a
sequence's length compute garbage that is masked to zero on output and
cannot contaminate earlier steps.
"""

import sys

for _p in ("/opt/trn_rl_repo",):
    if _p not in sys.path:
        sys.path.append(_p)

import numpy as np
import ml_dtypes

import concourse.bass as bass
import concourse.tile as tile
from concourse import bacc, mybir
from concourse.bass import IndirectOffsetOnAxis
from concourse.bass_utils import run_bass_kernel_spmd

F32 = mybir.dt.float32
BF16 = mybir.dt.bfloat16
FP8 = mybir.dt.float8e4
I32 = mybir.dt.int32
AF = mybir.ActivationFunctionType
ALU = mybir.AluOpType
DR = mybir.MatmulPerfMode.DoubleRow

# problem sizes (full / per-core)
B, T, V, E, H, TAGS = 128, 512, 50000, 256, 512, 64
NC = 8
BL = B // NC   # 16 sequences per core
G = 4 * H      # 2048 gate width
SH = 64.0      # fp8 h scale (|h|<=1 -> |h*SH| <= 64 < 240)

# permutation taking pytorch gate order i,f,g,o -> f,g,i,o (chain-critical
# gates in the A psum half, computed first)
_GATE_PERM = np.concatenate([
    np.arange(H, 2 * H), np.arange(2 * H, 3 * H), np.arange(0, H),
    np.arange(3 * H, 4 * H)])


def _build(nc, Tn=T, Bl=BL, TC=4, RC=4):
    """Emit the per-core program. Tn shrinkable for dev testing."""
    ntok = Bl * Tn
    nchunk = ntok // 128
    KE = E // 128       # k-chunks for layer-1 input proj
    KH2 = 2 * H // 128  # k-chunks for layer-2 input proj / classifier
    KH = H // 128       # k-chunks for recurrent
    assert ntok % 128 == 0

    # ---- dram I/O ----
    embq = nc.dram_tensor("embq", [V, E], FP8, kind="ExternalInput")
    xf_idx = nc.dram_tensor("xf_idx", [128, nchunk], I32, kind="ExternalInput")
    xb_idx = nc.dram_tensor("xb_idx", [128, nchunk], I32, kind="ExternalInput")
    rev128 = nc.dram_tensor("rev128", [128, nchunk], I32, kind="ExternalInput")
    mask32 = nc.dram_tensor("mask32", [32, Tn], F32, kind="ExternalInput")
    ident = nc.dram_tensor("ident", [128, 128], BF16, kind="ExternalInput")
    gsc = nc.dram_tensor("gsc", [32, 1], F32, kind="ExternalInput")
    sel2 = nc.dram_tensor("sel2", [32, 64], BF16, kind="ExternalInput")
    identq = nc.dram_tensor("identq", [128, 128], FP8, kind="ExternalInput")
    sxq = nc.dram_tensor("sxq", [128, 2], F32, kind="ExternalInput")
    pgsc = nc.dram_tensor("pgsc", [128, 4], F32, kind="ExternalInput")

    wih, whh, biasd = {}, {}, {}
    for s, din in (("f1", E), ("b1", E), ("f2", 2 * H), ("b2", 2 * H)):
        wih[s] = nc.dram_tensor(f"wihT_{s}", [din, G], FP8, kind="ExternalInput")
        whh[s] = nc.dram_tensor(f"whhT_{s}", [H, G], FP8, kind="ExternalInput")
        biasd[s] = nc.dram_tensor(f"bias_{s}", [128, G], F32, kind="ExternalInput")
    wcls = nc.dram_tensor("wclsT", [2 * H, TAGS], BF16, kind="ExternalInput")
    bcls = nc.dram_tensor("bcls", [TAGS, 1], F32, kind="ExternalInput")

    gx = {s: nc.dram_tensor(f"gx_{s}", [ntok, G], BF16)
          for s in ("f1", "b1", "f2", "b2")}
    # per-direction layer outputs; backward halves stay in scan order and are
    # un-reversed by the consumers' row gathers (no per-step scatters)
    hout = {s: nc.dram_tensor(f"hout_{s}", [ntok, H], BF16)
            for s in ("f1", "b1", "f2", "b2")}
    logitsT = nc.dram_tensor("logitsT", [TAGS, ntok], F32, kind="ExternalOutput")

    with tile.TileContext(nc) as tc:
        with tc.tile_pool(name="const", bufs=1) as cpool:
            def load_const(nm, shape, dt, src_ap):
                t = cpool.tile(shape, dt, name=nm, tag=nm)
                nc.gpsimd.dma_start(t[:], src_ap)
                return t

            xf_sb = load_const("xf_sb", [128, nchunk], I32, xf_idx[:])
            xb_sb = load_const("xb_sb", [128, nchunk], I32, xb_idx[:])
            rev128_sb = load_const("rev128_sb", [128, nchunk], I32, rev128[:])
            mask_sb = load_const("mask_sb", [32, Tn], F32, mask32[:])
            id_sb = load_const("id_sb", [128, 128], BF16, ident[:])
            gsc_sb = load_const("gsc_sb", [32, 1], F32, gsc[:])
            sel_sb = load_const("sel_sb", [32, 64], BF16, sel2[:])
            idq_sb = load_const("idq_sb", [128, 128], FP8, identq[:])
            sxq_sb = load_const("sxq_sb", [128, 2], F32, sxq[:])
            pgsc_sb = load_const("pgsc_sb", [128, 4], F32, pgsc[:])
            bcls_sb = load_const("bcls_sb", [TAGS, 1], F32, bcls[:])
            bias_sb = {s: load_const(f"bias_sb_{s}", [128, G], F32, biasd[s][:])
                       for s in ("f1", "b1", "f2", "b2")}
            wcls_sb = cpool.tile([128, KH2, TAGS], BF16, name="wcls_sb")
            for k in range(KH2):
                nc.gpsimd.dma_start(wcls_sb[:, k, :], wcls[128 * k:128 * (k + 1), :])

            # layer-1 projections pipelined into the layer-1 scans: the
            # first t-block of gx is produced up front, the rest co-emitted
            # inside the scan loop ahead of the consuming steps
            nblk = Tn // 128
            with tc.tile_pool(name="pw1", bufs=1) as wp1, \
                 tc.tile_pool(name="pg1", bufs=4) as gp1, \
                 tc.tile_pool(name="pgT1", bufs=3) as tp1, \
                 tc.tile_pool(name="pps1", bufs=1, space="PSUM") as pp1, \
                 tc.tile_pool(name="pout1", bufs=3) as op1:
                emit1 = _proj_phase(
                    nc, tc, nchunk, KE, wih=wih, bias_sb=bias_sb, gx=gx,
                    jobs=[("f1", embq, xf_sb, True), ("b1", embq, xb_sb, True)],
                    id_sb=idq_sb, sx_sb=sxq_sb[:, 0:1], pgsc_sb=pgsc_sb,
                    sidx0=0, psT_bufs=1, pps_bufs=1,
                    pools=(wp1, gp1, tp1, pp1, op1))
                if nblk >= 1:
                    order = [b * nblk + blk for blk in range(nblk)
                             for b in range(Bl)]
                    head = [c for c in order if c % nblk == 0]
                    rest = [c for c in order if c % nblk != 0]
                else:
                    head, rest = list(range(nchunk)), []
                for c in head:
                    emit1(c)

                def coemit1(t):
                    i = t // 6
                    if t % 6 == 0 and i < len(rest):
                        emit1(rest[i])

                _scan_phase(nc, tc, Tn, Bl, TC, RC, KH,
                            scans=("f1", "b1"), whh=whh, gx=gx, hout=hout,
                            mask_sb=mask_sb, id_sb=id_sb, gsc_sb=gsc_sb,
                            sel_sb=sel_sb, coemit=coemit1, psA_bufs=1)
            # layer-2 projections pipelined into the layer-2 scans (all
            # proj2 inputs are ready once the layer-1 scans finish): input
            # token (b,t) for the fwd scan is [f1h[t], s1h[rev(t)]]; for the
            # bwd scan it is [f1h[rev(t)], s1h[t]]
            with tc.tile_pool(name="pw2", bufs=1) as wp2, \
                 tc.tile_pool(name="pg2", bufs=4) as gp2, \
                 tc.tile_pool(name="pgT2", bufs=3) as tp2, \
                 tc.tile_pool(name="pps2", bufs=1, space="PSUM") as pp2, \
                 tc.tile_pool(name="pout2", bufs=3) as op2:
                emit2 = _proj_phase(
                    nc, tc, nchunk, KH2, wih=wih, bias_sb=bias_sb, gx=gx,
                    jobs=[("f2", (hout["f1"], None, hout["b1"], rev128_sb), None, False),
                          ("b2", (hout["f1"], rev128_sb, hout["b1"], None), None, False)],
                    id_sb=idq_sb, sx_sb=sxq_sb[:, 1:2], pgsc_sb=pgsc_sb,
                    sidx0=2, psT_bufs=1, pps_bufs=1,
                    pools=(wp2, gp2, tp2, pp2, op2))
                if nblk >= 1:
                    order2 = [b * nblk + blk for blk in range(nblk)
                              for b in range(Bl)]
                    head2 = [c for c in order2 if c % nblk == 0]
                    rest2 = [c for c in order2 if c % nblk != 0]
                else:
                    head2, rest2 = list(range(nchunk)), []
                for c in head2:
                    emit2(c)

                def coemit2(t):
                    i = t // 6
                    if t % 6 == 0 and i < len(rest2):
                        emit2(rest2[i])

                _scan_phase(nc, tc, Tn, Bl, TC, RC, KH,
                            scans=("f2", "b2"), whh=whh, gx=gx, hout=hout,
                            mask_sb=mask_sb, id_sb=id_sb, gsc_sb=gsc_sb,
                            sel_sb=sel_sb, coemit=coemit2, psA_bufs=1)

            # classifier: logits^T = W_cls @ out2^T + b_cls
            with tc.tile_pool(name="cls", bufs=3) as gp, \
                 tc.tile_pool(name="clsT", bufs=3) as gtp, \
                 tc.tile_pool(name="clsps", bufs=1, space="PSUM") as pp, \
                 tc.tile_pool(name="clso", bufs=3) as op:
                for c in range(nchunk):
                    o2 = gp.tile([128, 2 * H], BF16, tag="in")
                    nc.gpsimd.dma_start(o2[:, 0:H], hout["f2"][128 * c:128 * (c + 1), :])
                    nc.gpsimd.indirect_dma_start(
                        out=o2[:, H:2 * H], out_offset=None, in_=hout["b2"][:],
                        in_offset=IndirectOffsetOnAxis(ap=rev128_sb[:, c:c + 1], axis=0))
                    psT2 = pp.tile([128, KH2, 128], F32, tag="psT2",
                                   name="psT2", bufs=2)
                    for k in range(KH2):
                        nc.tensor.matmul(psT2[:, k, :], o2[:, 128 * k:128 * (k + 1)],
                                         id_sb[:], start=True, stop=True)
                    o2T = gtp.tile([128, KH2, 128], BF16, tag="inT")
                    nc.scalar.activation(o2T[:], psT2[:], AF.Copy)
                    ps = pp.tile([TAGS, 128], F32, name="clsps_t", tag="clsps",
                                 bufs=4)
                    for k in range(KH2):
                        nc.tensor.matmul(ps[:], wcls_sb[:, k, :], o2T[:, k, :],
                                         start=(k == 0), stop=(k == KH2 - 1))
                    lg = op.tile([TAGS, 128], F32, tag="lg")
                    nc.scalar.activation(lg[:], ps[:], AF.Identity,
                                         bias=bcls_sb[:, 0:1])
                    nc.gpsimd.dma_start(logitsT[:, 128 * c:128 * (c + 1)], lg[:])
    return nc


def _proj_phase(nc, tc, nchunk, KD, wih, bias_sb, gx, jobs, id_sb=None,
                sx_sb=None, pgsc_sb=None, sidx0=0, psT_bufs=2, pps_bufs=4,
                emitter_only=False, pools=None):
    """gx_s = input @ W_ih_s^T + b_s, written contiguously in scan-time order.

    fp8 path: inputs quantized to e4m3 (x sx), weights pre-quantized on host
    (x swi); DoubleRow matmuls pair k-chunks; the f32 psum is descaled by
    the per-scan pgsc AP and biased in one DVE scalar_tensor_tensor.

    jobs: (scan_name, dram_src, idx_tile_or_None, is_emb). For is_emb the idx
    tile holds embedding row ids (fp32 gather); otherwise rows of src are
    read contiguously (idx None) or gathered (idx set, layer-2 backward).
    """
    D = KD * 128

    def _emit(wsb, gpool, tpool, ppool, opool):
        def emit_chunk(c):
            for js, (s, dsrc, idx, is_emb) in enumerate(jobs):
                if is_emb:
                    xq = gpool.tile([128, D], FP8, tag="xq")
                    nc.gpsimd.indirect_dma_start(
                        out=xq[:], out_offset=None, in_=dsrc[:],
                        in_offset=IndirectOffsetOnAxis(ap=idx[:, c:c + 1], axis=0))
                else:
                    fsrc, fidx, bsrc, bidx = dsrc
                    xin = gpool.tile([128, D], BF16, tag="e16")
                    for src_t, sidx, lo in ((fsrc, fidx, 0), (bsrc, bidx, H)):
                        if sidx is None:
                            nc.gpsimd.dma_start(xin[:, lo:lo + H],
                                                src_t[128 * c:128 * (c + 1), :])
                        else:
                            nc.gpsimd.indirect_dma_start(
                                out=xin[:, lo:lo + H], out_offset=None, in_=src_t[:],
                                in_offset=IndirectOffsetOnAxis(ap=sidx[:, c:c + 1], axis=0))
                    xq = gpool.tile([128, D], FP8, tag="xq")
                    nc.scalar.activation(xq[:], xin[:], AF.Copy, scale=sx_sb)
                psT = ppool.tile([128, KD, 128], F32, tag="psT", name="psT",
                                 bufs=psT_bufs)
                for k in range(KD):
                    nc.tensor.matmul(psT[:, k, :], xq[:, 128 * k:128 * (k + 1)],
                                     id_sb[:], start=True, stop=True)
                xT = tpool.tile([128, KD, 128], FP8, tag="xT")
                nc.scalar.activation(xT[:], psT[:], AF.Copy)
                gout = opool.tile([128, G], BF16, tag="gout")
                for n in range(G // 512):
                    ps = ppool.tile([128, 512], F32, name="pps", tag="pps",
                                     bufs=pps_bufs)
                    for kp in range(KD // 2):
                        nc.tensor.matmul(
                            ps[:], xT[:, 2 * kp:2 * kp + 2, :],
                            wsb[s][:, 2 * kp:2 * kp + 2, 512 * n:512 * (n + 1)],
                            start=(kp == 0), stop=(kp == KD // 2 - 1),
                            perf_mode=DR)
                    nc.vector.scalar_tensor_tensor(
                        out=gout[:, 512 * n:512 * (n + 1)], in0=ps[:],
                        scalar=pgsc_sb[:, sidx0 + js:sidx0 + js + 1],
                        in1=bias_sb[s][:, 512 * n:512 * (n + 1)],
                        op0=ALU.mult, op1=ALU.add)
                nc.gpsimd.dma_start(gx[s][128 * c:128 * (c + 1), :], gout[:])
        return emit_chunk

    def _load_w(wpool):
        wsb = {}
        for s, _, _, _ in jobs:
            wsb[s] = wpool.tile([128, KD, G], FP8, tag=f"w{s}", name=f"wih_{s}")
            for k in range(KD):
                nc.gpsimd.dma_start(wsb[s][:, k, :],
                                    wih[s][128 * k:128 * (k + 1), :])
        return wsb

    if pools is not None:
        wpool, gpool, tpool, ppool, opool = pools
        return _emit(_load_w(wpool), gpool, tpool, ppool, opool)
    with tc.tile_pool(name="pw", bufs=1) as wpool, \
         tc.tile_pool(name="pg", bufs=4) as gpool, \
         tc.tile_pool(name="pgT", bufs=3) as tpool, \
         tc.tile_pool(name="pps", bufs=4, space="PSUM") as ppool, \
         tc.tile_pool(name="pout", bufs=3) as opool:
        emit_chunk = _emit(_load_w(wpool), gpool, tpool, ppool, opool)
        for c in range(nchunk):
            emit_chunk(c)


def _scan_phase(nc, tc, Tn, Bl, TC, RC, KH, scans, whh, gx, hout,
                mask_sb, id_sb, gsc_sb, sel_sb, coemit=None, psA_bufs=2):
    """Lane-packed dual-direction scan, block-diagonal DoubleRow.

    Forward lanes sit on partitions 0:16, backward on 16:32. Each recurrent
    fp8 DoubleRow matmul carries the two DIRECTIONS in its two k-tiles:
    stationary tile0 = [hT_f | 0], tile1 = [0 | hT_b] (built by selector-
    matrix transposes), moving tile0 = W_f chunk, tile1 = W_b chunk, so one
    m=32 matmul at PSUM partition 0 (the only base DoubleRow supports)
    accumulates both directions. Gate PSUM halves A (f,g) and B (i,o) are
    pre-filled with gx via id32 matmuls; activations descale via the gsc AP
    and are stored bf16; c stays f32; h is downcast to fp8 (x SH).
    """
    sf, sb = scans
    gxv = {s: gx[s].ap().rearrange("(b t) d -> b t d", b=Bl) for s in scans}
    houtv = {s: hout[s].ap().rearrange("(b t) d -> b t d", b=Bl) for s in scans}
    H2 = 2 * H
    with tc.tile_pool(name="sw", bufs=1) as wpool, \
         tc.tile_pool(name="sgx", bufs=3) as gxpool, \
         tc.tile_pool(name="sst", bufs=1) as stpool, \
         tc.tile_pool(name="sps", bufs=1, space="PSUM") as pspool, \
         tc.tile_pool(name="swk", bufs=3) as wkpool, \
         tc.tile_pool(name="shT", bufs=2) as htpool, \
         tc.tile_pool(name="srng", bufs=3) as rpool:
        # W pair layout: [:, k, 0, :] = W_f rows k-chunk, [:, k, 1, :] = W_b
        wsb = wpool.tile([128, KH, 2, G], FP8, tag="whhp", name="whhp")
        for k in range(KH):
            nc.gpsimd.dma_start(wsb[:, k, 0, :], whh[sf][128 * k:128 * (k + 1), :])
            nc.gpsimd.dma_start(wsb[:, k, 1, :], whh[sb][128 * k:128 * (k + 1), :])
        c_st = stpool.tile([32, H], F32, tag="c", name="c_st")
        nc.vector.memset(c_st[:], 0.0)
        hT = htpool.tile([128, KH, 2, 32], FP8, tag="hT", name="hT0")
        nc.vector.memset(hT[:], 0.0)
        gxc = {}   # (tt//TC) -> tile

        def load_gx(tt):
            t_ = gxpool.tile([32, TC, G], BF16, tag="gx", name="gxc")
            nc.sync.dma_start(t_[0:16, :, :], gxv[sf][:, tt:tt + TC, :])
            nc.sync.dma_start(t_[16:32, :, :], gxv[sb][:, tt:tt + TC, :])
            gxc[tt // TC] = t_

        def inject(tt, half_tag, lo):
            ps = pspool.tile([32, H2], F32, tag=half_tag, name=half_tag,
                             bufs=(psA_bufs if half_tag == "psA" else 1))
            src = gxc[tt // TC]
            for n in range(2):
                nc.tensor.matmul(
                    ps[:, 512 * n:512 * (n + 1)], id_sb[0:32, 0:32],
                    src[:, tt % TC, lo + 512 * n:lo + 512 * (n + 1)],
                    start=True, stop=False, skip_group_check=True)
            return ps

        load_gx(0)
        load_gx(TC)
        gA = inject(0, "psA", 0)
        gB = inject(0, "psB", H2)
        for t in range(Tn):
            # recurrent block-diagonal DR matmuls accumulate onto injected gx;
            # region order (f, g, i, o) releases chain-critical gates earliest
            def recur(half, coff, n):
                dst = half[:, 512 * n:512 * (n + 1)]
                for k in range(KH):
                    nc.tensor.matmul(
                        dst, hT[:, k, :, :],
                        wsb[:, k, :, coff + 512 * n:coff + 512 * (n + 1)],
                        start=False, stop=(k == KH - 1),
                        perf_mode=DR, skip_group_check=True)

            gact = wkpool.tile([32, G], BF16, tag="gact", name="gact")
            recur(gA, 0, 0)   # f gates
            nc.scalar.activation(gact[:, 0:H], gA[:, 0:H], AF.Sigmoid,
                                 scale=gsc_sb[:, 0:1])
            recur(gA, 0, 1)   # g gates
            nc.scalar.activation(gact[:, H:H2], gA[:, H:H2], AF.Tanh,
                                 scale=gsc_sb[:, 0:1])
            recur(gB, H2, 0)  # i gates
            nc.scalar.activation(gact[:, H2:3 * H], gB[:, 0:H], AF.Sigmoid,
                                 scale=gsc_sb[:, 0:1])
            recur(gB, H2, 1)  # o gates
            if t + 1 < Tn:
                # next step's gx lands in PSUM behind the o-region
                if (t + 1) % TC == 0 and t + 1 + TC < Tn:
                    load_gx(t + 1 + TC)
                gA_n = inject(t + 1, "psA", 0)
                gB_n = inject(t + 1, "psB", H2)
            t1 = wkpool.tile([32, H], F32, tag="t1", name="t1")
            t2 = wkpool.tile([32, H], BF16, tag="t2", name="t2")
            tch = wkpool.tile([32, H], BF16, tag="tch", name="tch")
            h16 = wkpool.tile([32, H], BF16, tag="h16", name="h16")
            if t % RC == 0:
                ring = rpool.tile([32, RC, H], BF16, tag="ring", name="ring")
            hT_ps = pspool.tile([128, KH, 2, 32], F32, tag="hT_ps",
                                name="hT_ps", bufs=1)
            hTn = htpool.tile([128, KH, 2, 32], FP8, tag="hT", name="hTn")
            nc.scalar.activation(gact[:, 3 * H:G], gB[:, H:H2], AF.Sigmoid,
                                 scale=gsc_sb[:, 0:1])
            # c/h tail pipelined in half-H chunks: the low half reaches the
            # next step's stationary operand while the high half trails
            HQ = H // 2
            for hh in range(2):
                q = slice(HQ * hh, HQ * (hh + 1))
                nc.vector.tensor_tensor(out=t1[:, q], in0=gact[:, HQ * hh:
                                                              HQ * (hh + 1)],
                                        in1=c_st[:, q], op=ALU.mult)
                nc.vector.tensor_tensor(out=t2[:, q],
                                        in0=gact[:, H2 + HQ * hh:
                                                 H2 + HQ * (hh + 1)],
                                        in1=gact[:, H + HQ * hh:
                                                 H + HQ * (hh + 1)],
                                        op=ALU.mult)
                nc.vector.tensor_tensor(out=c_st[:, q], in0=t1[:, q],
                                        in1=t2[:, q], op=ALU.add)
                nc.scalar.activation(tch[:, q], c_st[:, q], AF.Tanh)
            for hh in range(2):
                q = slice(HQ * hh, HQ * (hh + 1))
                nc.vector.tensor_tensor(out=h16[:, q],
                                        in0=gact[:, 3 * H + HQ * hh:
                                                 3 * H + HQ * (hh + 1)],
                                        in1=tch[:, q], op=ALU.mult)
                for k in (2 * hh, 2 * hh + 1):
                    nc.tensor.matmul(hT_ps[:, k, 0, :],
                                     h16[:, 128 * k:128 * (k + 1)],
                                     sel_sb[:, 0:32], start=True, stop=True)
                    nc.tensor.matmul(hT_ps[:, k, 1, :],
                                     h16[:, 128 * k:128 * (k + 1)],
                                     sel_sb[:, 32:64], start=True, stop=True)
                nc.vector.tensor_scalar_mul(hTn[:, 2 * hh:2 * hh + 2, :, :],
                                            hT_ps[:, 2 * hh:2 * hh + 2, :, :],
                                            SH)
            hT = hTn
            if t + 1 < Tn:
                gA, gB = gA_n, gB_n
            if coemit is not None:
                coemit(t)
            nc.gpsimd.tensor_scalar_mul(ring[:, t % RC, :], h16[:],
                                        mask_sb[:, t:t + 1])
            if (t + 1) % RC == 0:
                t0r = t + 1 - RC
                nc.gpsimd.dma_start(houtv[sf][:, t0r:t0r + RC, :],
                                    ring[0:16, :, :])
                nc.gpsimd.dma_start(houtv[sb][:, t0r:t0r + RC, :],
                                    ring[16:32, :, :])


def _prep_inputs(inputs, Tn=T, Bl=BL, ncores=NC):
    """Host-side sharding + weight preprocessing. Returns per-core in_maps."""
    x = np.asarray(inputs["x"]).astype(np.int32)
    lengths = np.asarray(inputs["lengths"]).astype(np.int32)
    emb = np.asarray(inputs["emb"], dtype=np.float32)
    ntok = Bl * Tn

    # global fp8 scales: W_hh pre-scaled to fill e4m3; gx pre-scaled by sw*sh
    wmax = max(np.abs(np.asarray(inputs[f"W_hh_{s}"], np.float32)).max()
               for s in ("f1", "b1", "f2", "b2"))
    sw = 240.0 / max(wmax, 1e-30)
    S = sw * SH

    id128 = np.eye(128, dtype=ml_dtypes.bfloat16)
    idq = np.eye(128, dtype=ml_dtypes.float8_e4m3)
    gsc = np.full((32, 1), 1.0 / S, np.float32)
    sel2 = np.zeros((32, 64), ml_dtypes.bfloat16)
    for j in range(16):
        sel2[j, j] = 1.0
        sel2[16 + j, 32 + 16 + j] = 1.0
    sx1 = 240.0 / max(float(np.abs(emb).max()), 1e-30)
    sx2 = 240.0   # layer-2 inputs are h values, |h| < 1
    sxq = np.zeros((128, 2), np.float32)
    sxq[:, 0] = sx1
    sxq[:, 1] = sx2
    embq = np.clip(emb * sx1, -240.0, 240.0).astype(ml_dtypes.float8_e4m3)
    com = {"embq": embq, "ident": id128, "identq": idq,
           "gsc": gsc, "sel2": sel2, "sxq": sxq}
    pg = np.zeros((128, 4), np.float32)
    for si, s in enumerate(("f1", "b1", "f2", "b2")):
        w_ih = np.asarray(inputs[f"W_ih_{s}"], np.float32)[_GATE_PERM]
        w_hh = np.asarray(inputs[f"W_hh_{s}"], np.float32)[_GATE_PERM]
        b = np.asarray(inputs[f"b_{s}"], np.float32)[_GATE_PERM]
        swi = 240.0 / max(float(np.abs(w_ih).max()), 1e-30)
        com[f"wihT_{s}"] = np.clip(
            np.ascontiguousarray(w_ih.T) * swi, -240.0, 240.0
        ).astype(ml_dtypes.float8_e4m3)
        sx = sx1 if si < 2 else sx2
        pg[:, si] = S / (sx * swi)
        com[f"whhT_{s}"] = np.clip(
            np.ascontiguousarray(w_hh.T) * sw, -240.0, 240.0
        ).astype(ml_dtypes.float8_e4m3)
        com[f"bias_{s}"] = np.tile((b * S).reshape(1, G), (128, 1))
    com["pgsc"] = pg
    com["wclsT"] = np.ascontiguousarray(
        np.asarray(inputs["W_cls"], np.float32).T).astype(ml_dtypes.bfloat16)
    com["bcls"] = np.asarray(inputs["b_cls"], np.float32).reshape(TAGS, 1)

    def chunked(a):  # [ntok] -> [128, ntok//128] with chunk c in column c
        return np.ascontiguousarray(a.reshape(-1).reshape(ntok // 128, 128).T)

    in_maps = []
    for c in range(ncores):
        xs = x[Bl * c:Bl * (c + 1), :Tn]
        ls = np.minimum(lengths[Bl * c:Bl * (c + 1)], Tn)
        ts = np.arange(Tn)[None, :]
        rev = np.where(ts < ls[:, None], ls[:, None] - 1 - ts, ts)  # [Bl,Tn]
        xrev = np.take_along_axis(xs, rev, axis=1)
        flat_rev = (np.arange(Bl)[:, None] * Tn + rev).astype(np.int32)
        mask = (ts < ls[:, None])
        m32 = np.zeros((32, Tn), np.float32)
        m32[0:16] = mask
        m32[16:32] = mask
        m = {
            "xf_idx": chunked(xs),
            "xb_idx": chunked(xrev),
            "rev128": chunked(flat_rev),
            "mask32": m32,
        }
        m.update(com)
        in_maps.append(m)
    return in_maps


_CACHED = {}


def kernel(**inputs) -> np.ndarray:
    if "nc" not in _CACHED:
        nc = bacc.Bacc("TRN2", target_bir_lowering=False, debug=False,
                       num_devices=NC)
        _build(nc)
        nc.compile()
        _CACHED["nc"] = nc
    nc = _CACHED["nc"]
    in_maps = _prep_inputs(inputs)
    res = run_bass_kernel_spmd(nc, in_maps, core_ids=list(range(NC)), trace=False)
    outs = []
    for c in range(NC):
        lt = res.results[c]["logitsT"]  # [TAGS, ntok]
        outs.append(np.ascontiguousarray(lt.T.reshape(BL, T, TAGS)))
    return np.concatenate(outs, axis=0).astype(np.float32)
